# revision 1
# baseline (speedup 1.0000x reference)
"""CustomBatchNorm2D forward on 8 Trainium2 NeuronCores — bf16 I/O,
2-tile-fused DMAs, sums on DVE, normalize on ACT + Pool.

Memory-regime kernel: keep the DMA engines saturated on the 8.4 MB/core
of bf16 traffic (~23.3 us modeled, ~21-24 us measured; f32 baseline was
46.6 us). Best-measured variant of the session (21.0 us):
  - x/out travel as bfloat16 (stats stay f32; rel err ~2.5e-3 vs the
    2e-2 gate) — halves the f32 baseline's traffic.
  - 16 DMAs/rep of 512 KB each: 32 unfused 256 KB DMAs make the SP
    sequencer + HWDGE descriptor-gen (~600 ns/DMA, shared) the critical
    path; 8 fully-fused 1 MB DMAs starve the pipeline at phase
    boundaries. 512 KB is the sweet spot.
  - Per-sample sums: one DVE reduce per fused tile via a [128, 2, 1024]
    access-pattern view; nothing else runs on DVE before the stats
    chain, so the chain starts as soon as the last load lands.
  - Normalize on ACT (activation, 3 of 4 tiles) + Pool (tensor_scalar,
    1 of 4): stores become ready as the DMA queue reaches them, never
    queued behind DVE's in-order sum backlog.
  - data pool bufs=2: next rep's loads use the other SBUF slot and
    don't wait on this rep's stores.
  - Emission order stays the natural [loads r][stores r]: reordering
    to [loads r+1][stores r] looked faster in the cost model but was
    ~2.5 us/rep slower on hardware.

Algorithm (T = sum_i t[i,j], Q = sum_i t[i,j]^2, t = per-sample sums):
    |diag[j]| = |T[j]^2/N - Q[j]| / HW
    out       = A[j]*x + B[j],  A = gamma*|diag|,  B = beta - A*T/(N*HW)

Sharding: channels C (512 -> 64 per core), no collective. Partition
p = quad*32 + ch; fused-tile column = blk*1024 + hw; sample
i = (f*2 + blk)*4 + quad. The quad fold (and the gamma/beta broadcast)
is one [128,128] matmul against a mod-32 selection matrix.
"""

import numpy as np
import ml_dtypes

import concourse.bacc as bacc
import concourse.mybir as mybir
import concourse.tile as tile
from concourse.bass_utils import run_bass_kernel_spmd

N, C, H, W = 32, 512, 32, 32
NCORES = 8
CPC = C // NCORES          # 64 channels per core
HW = H * W                 # 1024
CG = 2                     # channel groups per core
CPG = CPC // CG            # 32 channels per group
SPT = 128 // CPG           # 4 samples per (unfused) tile
NTG = N // SPT             # 8 unfused tiles per group
FU = 2                     # unfused tiles fused per DMA tile
F = NTG // FU              # 4 fused tiles per group
TW = FU * HW               # 2048 columns per fused tile
f32 = mybir.dt.float32
bf16 = mybir.dt.bfloat16
nbf16 = ml_dtypes.bfloat16

_CACHE = {}


def _build(reps=1):
    if reps in _CACHE:
        return _CACHE[reps]

    nc = bacc.Bacc(
        "TRN2",
        target_bir_lowering=False,
        debug=False,
        enable_asserts=False,
        num_devices=NCORES,
    )
    x = nc.dram_tensor("x", [CG, F, 128, TW], bf16, kind="ExternalInput")
    gamma = nc.dram_tensor("gamma", [CPC], f32, kind="ExternalInput")
    beta = nc.dram_tensor("beta", [CPC], f32, kind="ExternalInput")
    out = nc.dram_tensor("out", [CG, F, 128, TW], bf16, kind="ExternalOutput")

    AX = mybir.AxisListType.X
    MUL = mybir.AluOpType.mult
    ADD = mybir.AluOpType.add
    SUB = mybir.AluOpType.subtract
    AF = mybir.ActivationFunctionType

    with tile.TileContext(nc) as tc:
        with (
            tc.tile_pool(name="data", bufs=2) as dp,
            tc.tile_pool(name="psum", bufs=1, space="PSUM") as pp,
        ):
          # fold matrix M4[p,f] = 1.0 if p == f (mod 32): M4.T @ v sums
          # the four quad-slots, leaving the total in all of them
          w_i = nc.alloc_sbuf_tensor("w_i", [128, 128], mybir.dt.int32).ap()
          M4 = nc.alloc_sbuf_tensor("M4", [128, 128], f32).ap()
          nc.gpsimd.iota(w_i, pattern=[[-1, 128]], base=128, channel_multiplier=1)
          nc.vector.tensor_scalar(w_i, w_i, CPG - 1, None, mybir.AluOpType.bitwise_and)
          nc.vector.tensor_scalar(M4, w_i, 0, None, mybir.AluOpType.is_equal)

          # small per-group stats tensors, raw-allocated, shared across reps
          stats_t = {}
          for g in range(CG):
            stats_t[g] = {
                name: nc.alloc_sbuf_tensor(f"{name}{g}", [128, w], f32).ap()
                for name, w in [
                    ("ST", 4), ("STf", 4), ("t", NTG), ("sq8", NTG),
                    ("mneg", 1), ("gmneg", 1), ("u", 1),
                    ("au", 1), ("A", 1), ("B", 1),
                ]
            }

          for _rep in range(reps):
            # every load up front so the sync-ring FIFO is
            # [A loads][B loads][A stores][B stores] with no idle slots
            xtiles = {}
            for g in range(CG):
                for f in range(F):
                    xt = dp.tile([128, TW], bf16, name=f"x{g}_{f}", tag=f"x{g}_{f}")
                    nc.sync.dma_start(xt, x[g, f])
                    xtiles[g, f] = xt

            # stats tile cols: [T, Q, gamma, beta]; gamma/beta sit in
            # quad-slot 0 with the rest zeroed, so the fold matmul also
            # broadcasts them to all slots
            for g in range(CG):
                ST = stats_t[g]["ST"]
                nc.gpsimd.memset(ST[:, 2:4], 0.0)
                sl = slice(g * CPG, (g + 1) * CPG)
                nc.scalar.dma_start(ST[0:CPG, 2:3], gamma[sl][:, None])
                nc.scalar.dma_start(ST[0:CPG, 3:4], beta[sl][:, None])

            for g in range(CG):
                st = stats_t[g]
                # per-sample channel sums: one DVE reduce per fused tile
                # ([128, FU, 1024] view -> FU columns of t_g)
                t_g = st["t"]
                for f in range(F):
                    xt = xtiles[g, f]
                    q = f * FU
                    nc.vector.reduce_sum(
                        t_g[:, q : q + FU],
                        xt.rearrange("p (b c) -> p b c", b=FU),
                        axis=AX,
                    )

                # T (col 0) and Q (col 1) totals over the 8 tile columns
                # (squares are per-sample, before any cross-sample fold)
                ST = st["ST"]
                sq8 = st["sq8"]
                nc.vector.reduce_sum(ST[:, 0:1], t_g[:, :], axis=AX)
                nc.vector.tensor_mul(sq8, t_g[:, :], t_g[:, :])
                nc.vector.reduce_sum(ST[:, 1:2], sq8[:, :], axis=AX)

                # fold the four quad-slots on the tensor engine; PSUM can
                # feed only one input per op, so copy to SBUF once
                STp = pp.tile([128, 4], f32, name=f"STp{g}", tag=f"STp{g}")
                nc.tensor.matmul(STp, M4, ST, start=True, stop=True)
                STf = st["STf"]
                nc.vector.tensor_copy(STf, STp)
                T = STf[:, 0:1]
                Q = STf[:, 1:2]
                gt = STf[:, 2:3]
                bt = STf[:, 3:4]

                # A = gamma*|T^2/N - Q|/HW ; B = beta + |..|*gamma*(-T/(N*HW))
                mneg, gmneg = st["mneg"], st["gmneg"]
                u, au, A, B = st["u"], st["au"], st["A"], st["B"]
                nc.vector.tensor_scalar(u, T, T[:, 0:1], None, MUL)
                nc.vector.scalar_tensor_tensor(u, u, 1.0 / N, Q, MUL, SUB)
                nc.vector.tensor_scalar_mul(mneg, T, -1.0 / (N * HW))
                nc.vector.tensor_mul(gmneg, gt, mneg)
                nc.scalar.activation(au, u, AF.Abs, scale=1.0 / HW)
                nc.vector.tensor_mul(A, au, gt)
                nc.scalar.activation(
                    B, au, AF.Identity, bias=bt[:, 0:1], scale=gmneg[:, 0:1]
                )

                # normalize whole fused tiles in place, rotated across
                # ACT / DVE / Pool, and store
                for f in range(F):
                    xt = xtiles[g, f]
                    if f == 1:
                        nc.gpsimd.tensor_scalar(
                            xt[:, :], xt[:, :], A[:, 0:1], B[:, 0:1], MUL, ADD
                        )
                    else:
                        nc.scalar.activation(
                            xt[:, :], xt[:, :], AF.Identity,
                            bias=B[:, 0:1], scale=A[:, 0:1],
                        )
                    nc.sync.dma_start(out[g, f], xt)

    nc.compile()
    _CACHE[reps] = nc
    return nc


def _in_maps(x, gamma, beta):
    x = np.ascontiguousarray(x, dtype=np.float32)
    gamma = np.ascontiguousarray(gamma, dtype=np.float32)
    beta = np.ascontiguousarray(beta, dtype=np.float32)
    maps = []
    for k in range(NCORES):
        sl = slice(k * CPC, (k + 1) * CPC)
        # [N, CPC, H, W] -> [CG, F, 128, TW] with sample
        # i = (f*FU + blk)*SPT + quad, partition p = quad*CPG + ch,
        # column = blk*HW + hw
        xk = x[:, sl].reshape(F, FU, SPT, CG, CPG, HW)
        xk = np.ascontiguousarray(
            xk.transpose(3, 0, 2, 4, 1, 5)  # (CG, F, SPT, CPG, FU, HW)
        ).reshape(CG, F, 128, TW).astype(nbf16)
        maps.append(
            {
                "x": xk,
                "gamma": np.ascontiguousarray(gamma[sl]),
                "beta": np.ascontiguousarray(beta[sl]),
            }
        )
    return maps


def _unshard(res):
    outs = []
    for k in range(NCORES):
        ok = res.results[k]["out"].astype(np.float32).reshape(
            CG, F, SPT, CPG, FU, HW
        )
        ok = ok.transpose(1, 4, 2, 0, 3, 5).reshape(N, CPC, H, W)
        outs.append(ok)
    return np.ascontiguousarray(np.concatenate(outs, axis=1))


def run(x, gamma, beta, trace=False, **kw):
    """Run on hardware; returns (full_output, BassKernelResults)."""
    nc = _build()
    res = run_bass_kernel_spmd(
        nc, _in_maps(x, gamma, beta), list(range(NCORES)), trace=trace, **kw
    )
    return _unshard(res), res


def kernel(x, gamma, beta):
    out, _ = run(x, gamma, beta)
    return out



# revision 10
# speedup vs baseline: 1.7928x; 1.7928x over previous
"""CustomBatchNorm2D forward on 8 Trainium2 NeuronCores — bf16 I/O,
sums on DVE+ACT, stats chain on Pool+PE, normalize on DVE (4x mode).

Memory-regime kernel. Per core: 64 channels x 32 samples x 1024 hw as
8 fused tiles [128, 2048] bf16 (512 KB DMAs, 1456 ns each modeled).
The previous 2-group ACT/Pool-normalize kernel modeled 40.9 us
single-shot: DVE reduces (2194 ns/tile, in-order) couldn't keep up
with the load stream and blocked the tiny stats ops queued behind
them, so the first store left at 27.6 us. This version:

  - CG=4 channel groups of 16 channels (8 sample-slots on partitions):
    each group's stats need only 2 tiles, so A/B come out early and
    stores are all ready well before the load stream ends.
  - Per-sample sums: tile f=0 of each group via one DVE TensorReduce
    ([128, 2, 1024] view, 2194 ns); tile f=1 via two ACT Copy+accum_out
    halves (1225 ns each). DVE and ACT each consume sums faster than
    the 1456 ns/tile DMA delivery rate.
  - The tiny stats chain (square/fold/|diag|/A/B) runs on Pool + one PE
    fold-matmul + one 69 ns DVE mini-reduce, interleaved so nothing
    waits behind a big op in an in-order queue (see emission plan in
    the rep loop).
  - Normalize: all 8 tiles on DVE TensorScalarPtr (bf16 in/out, f32
    per-partition scalars) — 4x DVE mode, 594 ns/tile vs 1892 on ACT /
    2939 on Pool.
  - gamma/beta ride ONE packed [16, 4, 2] DMA emitted after the 8 load
    dma_starts (HWDGE descriptor-gen is shared and costs ~630 ns per
    DMA, so small DMAs must not precede the load descriptors).

Algorithm (T = sum_i t[i,c], Q = sum_i t[i,c]^2, t = per-sample sums):
    diag[c] = (Q[c] - T[c]^2/N) / HW   (sign irrelevant, abs applied)
    out     = A[c]*x + B[c],  A = gamma*|diag|,  B = beta - A*T/(N*HW)

Sharding: channels C (512 -> 64 per core), no collective. Partition
p = slot*16 + ch; fused-tile column = blk*1024 + hw; sample
i = (f*2 + blk)*8 + slot. The slot fold (and the gamma/beta broadcast)
is one [128,128] matmul against a mod-16 selection matrix.
"""

import numpy as np
import ml_dtypes

import concourse.bacc as bacc
import concourse.mybir as mybir
import concourse.tile as tile
from concourse.bass_utils import run_bass_kernel_spmd

N, C, H, W = 32, 512, 32, 32
NCORES = 8
CPC = C // NCORES          # 64 channels per core
HW = H * W                 # 1024
CG = 4                     # channel groups per core
CPG = CPC // CG            # 16 channels per group
SPT = 128 // CPG           # 8 sample slots per tile
NTG = N // SPT             # 4 unfused tiles per group
FU = 2                     # unfused tiles fused per DMA tile
F = NTG // FU              # 2 fused tiles per group
TW = FU * HW               # 2048 columns per fused tile
f32 = mybir.dt.float32
bf16 = mybir.dt.bfloat16
nbf16 = ml_dtypes.bfloat16

_CACHE = {}


def _build(reps=1):
    if reps in _CACHE:
        return _CACHE[reps]

    nc = bacc.Bacc(
        "TRN2",
        target_bir_lowering=False,
        debug=False,
        enable_asserts=False,
        num_devices=NCORES,
    )
    x = nc.dram_tensor("x", [CG, F, 128, TW], bf16, kind="ExternalInput")
    gb = nc.dram_tensor("gb", [CPG, CG, 2], f32, kind="ExternalInput")
    out = nc.dram_tensor("out", [CG, F, 128, TW], bf16, kind="ExternalOutput")

    AX = mybir.AxisListType.X
    MUL = mybir.AluOpType.mult
    ADD = mybir.AluOpType.add
    SUB = mybir.AluOpType.subtract
    AF = mybir.ActivationFunctionType

    with tile.TileContext(nc) as tc:
        with (
            tc.tile_pool(name="data", bufs=2) as dp,
            tc.tile_pool(name="psum", bufs=2, space="PSUM") as pp,
        ):
          # fold matrix M[p,f] = 1.0 if p == f (mod 16): M.T @ v sums the
          # eight slot replicas, leaving the total in all of them
          w_i = nc.alloc_sbuf_tensor("w_i", [128, 128], mybir.dt.int32).ap()
          M4 = nc.alloc_sbuf_tensor("M4", [128, 128], f32).ap()
          nc.gpsimd.iota(w_i, pattern=[[-1, 128]], base=128, channel_multiplier=1)
          nc.vector.tensor_scalar(w_i, w_i, CPG - 1, None, mybir.AluOpType.bitwise_and)
          nc.vector.tensor_scalar(M4, w_i, 0, None, mybir.AluOpType.is_equal)

          # ST[:, 4g:4g+2] = folded-input [T, Q] for group g (per rep);
          # ST[:, 4g+2:4g+4] = gamma/beta at slot 0 (loaded once)
          ST = nc.alloc_sbuf_tensor("ST", [128, 4 * CG], f32).ap()
          nc.gpsimd.memset(ST, 0.0)
          # ACT sum-op scratch output (values unused, accum_out is the point)
          scr = nc.alloc_sbuf_tensor("scr", [128, HW], bf16).ap()

          stats_t = {}
          for g in range(CG):
            stats_t[g] = {
                name: nc.alloc_sbuf_tensor(f"{name}{g}", [128, w], f32).ap()
                for name, w in [
                    ("ts", 2 * F * FU), ("STf", 4), ("f1", 4),
                    ("t2", 1), ("u", 1), ("qh", 1), ("d", 1),
                    ("nd", 1), ("ad", 1),
                    ("A", 1), ("tmp", 1), ("B", 1),
                ]
            }

          for _rep in range(reps):
            # every load up front so the DMA ring is [loads][gb][stores]
            xtiles = {}
            for g in range(CG):
                for f in (1, 0):  # ACT's tile (f=1) first: its two serial
                    # half-sums start ~1.5 us earlier per group
                    xt = dp.tile([128, TW], bf16, name=f"x{g}_{f}", tag=f"x{g}_{f}")
                    nc.sync.dma_start(xt, x[g, f])
                    xtiles[g, f] = xt

            if _rep == 0:
                # one packed gamma/beta DMA: [CPG, CG, 2] -> slot-0 rows of
                # every group's gamma/beta columns (ST viewed [128, CG, 4])
                ST3 = ST.rearrange("p (g c) -> p g c", g=CG)
                nc.scalar.dma_start(ST3[0:CPG, :, 2:4], gb[:, :, :])

            def emit_sums(g):
                # tile f=0 on DVE (one fused reduce), tile f=1 on ACT as
                # two half-tile Copy+accum ops; ts cols = [t0..t3|sq0..sq3]
                ts = stats_t[g]["ts"]
                nc.vector.reduce_sum(
                    ts[:, 0:2],
                    xtiles[g, 0].rearrange("p (b c) -> p b c", b=FU),
                    axis=AX,
                )
                for h in range(FU):
                    nc.scalar.activation(
                        scr, xtiles[g, 1][:, h * HW : (h + 1) * HW],
                        AF.Copy, accum_out=ts[:, 2 + h : 3 + h],
                    )

            def emit_chain(g):
                # [T,Q] via two strided Pool adds (keeps DVE free for the
                # big reduces + norms), slot-fold + gamma/beta broadcast on
                # PE, A/B chain on Pool
                st = stats_t[g]
                ts = st["ts"]
                STg = ST[:, 4 * g : 4 * g + 4]
                nc.gpsimd.tensor_mul(ts[:, 4:8], ts[:, 0:4], ts[:, 0:4])
                tsv = ts.rearrange("p (a c) -> p a c", a=2)
                f1 = st["f1"]  # [t02, t13, q02, q13]
                nc.gpsimd.tensor_add(
                    f1.rearrange("p (a c) -> p a c", a=2),
                    tsv[:, :, 0:2], tsv[:, :, 2:4],
                )
                f1v = f1.rearrange("p (c a) -> p c a", c=2)
                nc.gpsimd.tensor_add(STg[:, 0:2], f1v[:, :, 0:1], f1v[:, :, 1:2])
                STp = pp.tile([128, 4], f32, name=f"STp{g}", tag=f"STp{g}")
                nc.tensor.matmul(STp, M4, STg, start=True, stop=True)
                STf = st["STf"]
                # Pool cannot read PSUM; ACT does the PSUM->SBUF copy
                nc.scalar.copy(STf, STp)
                T = STf[:, 0:1]
                Q = STf[:, 1:2]
                gt = STf[:, 2:3]
                bt = STf[:, 3:4]
                # A = gamma*|Q - T^2/N|/HW ; B = beta - A*T/(N*HW)
                t2, u, qh, d = st["t2"], st["u"], st["qh"], st["d"]
                nd, ad = st["nd"], st["ad"]
                A, tmp, B = st["A"], st["tmp"], st["B"]
                nc.gpsimd.tensor_scalar_mul(t2, T, 1.0 / (N * HW))
                nc.gpsimd.tensor_mul(u, t2, T)
                nc.gpsimd.tensor_scalar_mul(qh, Q, 1.0 / HW)
                nc.gpsimd.tensor_sub(d, qh, u)
                nc.gpsimd.tensor_scalar_mul(nd, d, -1.0)
                nc.gpsimd.tensor_scalar(ad, d, nd[:, 0:1], None, mybir.AluOpType.max)
                nc.gpsimd.tensor_mul(A, ad, gt)
                nc.gpsimd.tensor_mul(tmp, A, t2)
                nc.gpsimd.tensor_sub(B, bt, tmp)

            def emit_norms_stores(g):
                st = stats_t[g]
                for f in range(F):
                    xt = xtiles[g, f]
                    nc.vector.tensor_scalar(
                        xt, xt, st["A"][:, 0:1], st["B"][:, 0:1], MUL, ADD
                    )
                    nc.sync.dma_start(out[g, f], xt)

            # Emission plan (engine in-order queues):
            #   DVE : s0 s1 | n0a n0b | s2 | n1a n1b | s3 | n2a n2b n3a n3b
            #   ACT : h00 h01 h10 h11 h20 h21 h30 h31
            #   Pool: chain0 | chain1 | chain2 | chain3
            # Group g's norms are emitted right after s(g+1) so the first
            # store is ready the moment the load stream drains, while the
            # big reduces never wait behind a not-yet-ready norm.
            emit_sums(0)
            emit_chain(0)
            emit_sums(1)
            emit_chain(1)
            emit_norms_stores(0)
            emit_sums(2)
            emit_chain(2)
            emit_norms_stores(1)
            emit_sums(3)
            emit_chain(3)
            emit_norms_stores(2)
            emit_norms_stores(3)

    nc.compile()
    _CACHE[reps] = nc
    return nc


def _in_maps(x, gamma, beta):
    x = np.ascontiguousarray(x, dtype=np.float32)
    gamma = np.ascontiguousarray(gamma, dtype=np.float32)
    beta = np.ascontiguousarray(beta, dtype=np.float32)
    maps = []
    for k in range(NCORES):
        sl = slice(k * CPC, (k + 1) * CPC)
        # [N, CPC, H, W] -> [CG, F, 128, TW] with sample
        # i = (f*FU + blk)*SPT + slot, partition p = slot*CPG + ch,
        # column = blk*HW + hw
        xk = x[:, sl].reshape(F, FU, SPT, CG, CPG, HW)
        xk = np.ascontiguousarray(
            xk.transpose(3, 0, 2, 4, 1, 5)  # (CG, F, SPT, CPG, FU, HW)
        ).reshape(CG, F, 128, TW).astype(nbf16)
        gbk = np.stack(
            [gamma[sl].reshape(CG, CPG).T, beta[sl].reshape(CG, CPG).T],
            axis=-1,
        )  # [CPG, CG, 2]
        maps.append({"x": xk, "gb": np.ascontiguousarray(gbk)})
    return maps


def _unshard(res):
    outs = []
    for k in range(NCORES):
        ok = res.results[k]["out"].astype(np.float32).reshape(
            CG, F, SPT, CPG, FU, HW
        )
        ok = ok.transpose(1, 4, 2, 0, 3, 5).reshape(N, CPC, H, W)
        outs.append(ok)
    return np.ascontiguousarray(np.concatenate(outs, axis=1))


def run(x, gamma, beta, trace=False, **kw):
    """Run on hardware; returns (full_output, BassKernelResults)."""
    nc = _build()
    res = run_bass_kernel_spmd(
        nc, _in_maps(x, gamma, beta), list(range(NCORES)), trace=trace, **kw
    )
    return _unshard(res), res


def kernel(x, gamma, beta):
    out, _ = run(x, gamma, beta)
    return out
